# revision 14
# baseline (speedup 1.0000x reference)
"""Trainium2 Bass kernel for nn_Evolution_26697516712465 (deep-snake GNN).

Self-contained: takes FULL inputs, shards batch across 8 NeuronCores internally
(one image per core; each core runs the snake for the polys of its own image),
returns FULL output [128, 128, 2] fp32.
"""
import numpy as np
import ml_dtypes
from contextlib import ExitStack

import concourse.bass as bass
import concourse.bacc as bacc
import concourse.mybir as mybir
import concourse.tile as tile
from concourse.library_config import mlp as mlp_lib
from concourse.bass_utils import run_bass_kernel_spmd

N_CORES = 8
B, C_IN, H, W = 8, 66, 128, 128
NP, V = 128, 128
RO = 4.0
DIL = (1, 1, 1, 2, 2, 4, 4)
NRES = 7
HW = H * W          # 16384
PADW = W + 2        # 130
PIMG = PADW * PADW  # 16900
PADV = 160          # 16 + 128 + 16 circular pad

f32 = mybir.dt.float32
bf16 = mybir.dt.bfloat16
i16 = mybir.dt.int16
AF = mybir.ActivationFunctionType
ALU = mybir.AluOpType

BF = ml_dtypes.bfloat16


def _bcast(ap_obj, n):
    """Append a step-0 (broadcast) innermost free dim of size n to an AP."""
    return bass.AP(tensor=ap_obj.tensor, offset=ap_obj.offset,
                   ap=[*ap_obj.ap, [0, n]])


def build_nc(P):
    """Build the SPMD Bass program. P = max polys per image."""
    nc = bacc.Bacc("TRN2", target_bir_lowering=False, debug=False)
    NV = P * 128           # corner-gather idx count (multiple of 128)
    ICOLS = NV // 16
    PADQ = -(-P // 4) * 4  # snake poly slots (multiple of 4)
    NQB = PADQ // 4

    # ---------------- inputs ----------------
    d_stack0 = nc.declare_dram_parameter("stack0", [128, PIMG], bf16, isOutput=False)
    d_stack1 = nc.declare_dram_parameter("stack1", [70, PIMG], bf16, isOutput=False)
    d_w1p0 = nc.declare_dram_parameter("w1p0", [128, 3, 2, 128], bf16, isOutput=False)
    d_w1p1 = nc.declare_dram_parameter("w1p1", [70, 3, 2, 128], bf16, isOutput=False)
    d_w2t = nc.declare_dram_parameter("w2t", [128, 2, 64], bf16, isOutput=False)
    d_pb0 = nc.declare_dram_parameter("pb0", [128, 2], f32, isOutput=False)
    d_fusb = nc.declare_dram_parameter("fusb", [128, 2], f32, isOutput=False)
    d_idxc = nc.declare_dram_parameter("idxc", [128, 4, ICOLS], i16, isOutput=False)
    d_wcomp = nc.declare_dram_parameter("wcomp", [128, 4, P], f32, isOutput=False)
    d_b2s = nc.declare_dram_parameter("b2s", [128, P, 64], f32, isOutput=False)
    d_coords = nc.declare_dram_parameter("coords", [128, P, 2], bf16, isOutput=False)
    d_iidx = nc.declare_dram_parameter("iidx", [128, PADQ * PADV // 16], i16, isOutput=False)
    d_base = nc.declare_dram_parameter("base", [128, PADQ, 2], f32, isOutput=False)
    d_headw = nc.declare_dram_parameter("headw", [66, 9, 128], bf16, isOutput=False)
    d_headb = nc.declare_dram_parameter("headb", [128, 3], f32, isOutput=False)
    d_resw = nc.declare_dram_parameter("resw", [128, 63, 128], bf16, isOutput=False)
    d_resb = nc.declare_dram_parameter("resb", [128, 3, 7], f32, isOutput=False)
    d_fusw = nc.declare_dram_parameter("fusw", [128, 8, 256], bf16, isOutput=False)
    d_pw1 = nc.declare_dram_parameter("pw1", [128, 10, 256], bf16, isOutput=False)
    d_pb1 = nc.declare_dram_parameter("pb1", [128, 2], f32, isOutput=False)
    d_pw2 = nc.declare_dram_parameter("pw2", [128, 2, 64], bf16, isOutput=False)
    d_pb2 = nc.declare_dram_parameter("pb2", [64, 1], f32, isOutput=False)
    d_pw3 = nc.declare_dram_parameter("pw3", [64, 2], bf16, isOutput=False)
    d_out = nc.declare_dram_parameter("out", [128, PADQ, 2], f32, isOutput=True)

    feat_dram = nc.dram_tensor("feat_dram", [HW, 64], f32)
    cc_in = nc.dram_tensor("cc_in", [NV, 128], bf16)

    with tile.TileContext(nc, num_cores=N_CORES) as tc, ExitStack() as top:
        wpool = top.enter_context(tc.tile_pool(name="weights", bufs=1))
        w2t_t = wpool.tile([128, 2, 64], bf16)
        nc.sync.dma_start(out=w2t_t, in_=d_w2t[:, :, :])
        pb0_t = wpool.tile([128, 2], f32)
        nc.sync.dma_start(out=pb0_t, in_=d_pb0[:, :])
        fusb_t = wpool.tile([128, 2], f32)
        nc.sync.dma_start(out=fusb_t, in_=d_fusb[:, :])
        idxc_t = wpool.tile([128, 4, ICOLS], i16)
        nc.sync.dma_start(out=idxc_t, in_=d_idxc[:, :, :])
        wcomp_t = wpool.tile([128, 4, P], f32)
        nc.sync.dma_start(out=wcomp_t, in_=d_wcomp[:, :, :])
        b2s_t = wpool.tile([128, P, 64], f32)
        nc.sync.dma_start(out=b2s_t, in_=d_b2s[:, :, :])
        coords_t = wpool.tile([128, P, 2], bf16)
        nc.sync.dma_start(out=coords_t, in_=d_coords[:, :, :])
        iidx_t = wpool.tile([128, PADQ * PADV // 16], i16)
        nc.sync.dma_start(out=iidx_t, in_=d_iidx[:, :])
        base_t = wpool.tile([128, PADQ, 2], f32)
        nc.sync.dma_start(out=base_t, in_=d_base[:, :, :])
        headw_t = wpool.tile([66, 9, 128], bf16)
        nc.sync.dma_start(out=headw_t, in_=d_headw[:, :, :])
        headb_t = wpool.tile([128, 3], f32)
        nc.sync.dma_start(out=headb_t, in_=d_headb[:, :])
        resw_t = wpool.tile([128, 63, 128], bf16)
        nc.sync.dma_start(out=resw_t, in_=d_resw[:, :, :])
        resb_t = wpool.tile([128, 3, 7], f32)
        nc.sync.dma_start(out=resb_t, in_=d_resb[:, :, :])
        fusw_t = wpool.tile([128, 8, 256], bf16)
        nc.sync.dma_start(out=fusw_t, in_=d_fusw[:, :, :])
        pw1_t = wpool.tile([128, 10, 256], bf16)
        nc.sync.dma_start(out=pw1_t, in_=d_pw1[:, :, :])
        pb1_t = wpool.tile([128, 2], f32)
        nc.sync.dma_start(out=pb1_t, in_=d_pb1[:, :])
        pw2_t = wpool.tile([128, 2, 64], bf16)
        nc.sync.dma_start(out=pw2_t, in_=d_pw2[:, :, :])
        pb2_t = wpool.tile([64, 1], f32)
        nc.sync.dma_start(out=pb2_t, in_=d_pb2[:, :])
        pw3_t = wpool.tile([64, 2], bf16)
        nc.sync.dma_start(out=pw3_t, in_=d_pw3[:, :])

        nc.gpsimd.load_library(mlp_lib)

        # relu1 lives across conv1 + conv2
        with tc.tile_pool(name="relu1", bufs=1) as rpool:
            r1 = [rpool.tile([128, HW], bf16, tag=f"r1_{m}", name=f"r1_{m}")
                  for m in range(2)]

            # ------------ conv1: 3x3 66->256 (bf16, K packed 128+70) ------------
            with tc.tile_pool(name="stacks", bufs=1) as stpool, \
                 tc.tile_pool(name="psumA", bufs=3, space="PSUM") as ppA:
                st0 = stpool.tile([128, PIMG], bf16)
                nc.sync.dma_start(out=st0, in_=d_stack0[:, :])
                st1 = stpool.tile([70, PIMG], bf16)
                nc.sync.dma_start(out=st1, in_=d_stack1[:, :])
                w1p0_t = stpool.tile([128, 3, 2, 128], bf16)
                nc.sync.dma_start(out=w1p0_t, in_=d_w1p0[:, :, :, :])
                w1p1_t = stpool.tile([70, 3, 2, 128], bf16)
                nc.sync.dma_start(out=w1p1_t, in_=d_w1p1[:, :, :, :])

                for t in range(32):          # hw tiles of 512 = 4 image rows
                    y0 = 4 * t
                    for m in range(2):       # out-channel half
                        ps = ppA.tile([128, 512], f32, tag="psA", name="psA")
                        i = 0
                        for (stk, wt) in ((st0, w1p0_t), (st1, w1p1_t)):
                            for kw in range(3):
                                rhs = bass.AP(tensor=stk.tensor,
                                              offset=stk.offset + y0 * PADW + kw,
                                              ap=[stk.ap[0], [PADW, 4], [1, 128]])
                                nc.tensor.matmul(ps, wt[:, kw, m, :], rhs,
                                                 start=(i == 0), stop=(i == 5))
                                i += 1
                        nc.scalar.activation(r1[m][:, t * 512:(t + 1) * 512], ps,
                                             AF.Relu, bias=pb0_t[:, m:m + 1])

            # ------------ conv2: 1x1 256->64, out [hw, 64] fp32 -> DRAM ------------
            with tc.tile_pool(name="psumB", bufs=2, space="PSUM") as ppB, \
                 tc.tile_pool(name="stage", bufs=3) as spool:
                for g in range(16):
                    ps2 = ppB.tile([128, 512], f32, tag="psB", name="psB")
                    for j in range(8):
                        hw0 = (g * 8 + j) * 128
                        for ch in range(2):
                            nc.tensor.matmul(ps2[:, j * 64:(j + 1) * 64],
                                             r1[ch][:, hw0:hw0 + 128],
                                             w2t_t[:, ch, :],
                                             start=(ch == 0), stop=(ch == 1))
                    stg = spool.tile([128, 512], f32, tag="stage", name="stg")
                    nc.vector.tensor_copy(stg, ps2)
                    dst = bass.AP(tensor=feat_dram, offset=g * 65536,
                                  ap=[[512, 128], [1, 512]])
                    nc.sync.dma_start(out=dst, in_=stg)

        # ------------ bilinear gather + weighted sum + vertex rows ------------
        with tc.tile_pool(name="gpool", bufs=1) as gpool:
            gts = []
            for c in range(4):
                gt = gpool.tile([128, P, 64], f32, tag=f"g{c}", name=f"g{c}")
                src = bass.AP(tensor=feat_dram, offset=0, ap=[[64, HW], [1, 64]])
                nc.gpsimd.dma_gather(gt, src, idxc_t[:, c, :], NV, NV, 64,
                                     single_packet=False)
                gts.append(gt)
            vert = gpool.tile([128, P, 64], f32, tag="vert", name="vert")
            tmp = gpool.tile([128, P, 64], f32, tag="tmp", name="tmp")
            for c in range(4):
                wb = _bcast(wcomp_t[:, c, :], 64)
                if c == 0:
                    nc.vector.tensor_tensor(vert, gts[c], wb, ALU.mult)
                else:
                    nc.vector.tensor_tensor(tmp, gts[c], wb, ALU.mult)
                    nc.vector.tensor_tensor(vert, vert, tmp, ALU.add)
            nc.vector.tensor_tensor(vert, vert, b2s_t, ALU.add)

            contrib = gpool.tile([128, P, 128], bf16, tag="contrib", name="contrib")
            nc.vector.memset(contrib, 0.0)
            nc.vector.tensor_copy(contrib[:, :, 0:64], vert)
            nc.vector.tensor_copy(contrib[:, :, 64:66], coords_t)
            # SBUF [v, q, ch] -> DRAM row q*128+v
            dst = bass.AP(tensor=cc_in, offset=0,
                          ap=[[128, 128], [128 * 128, P], [1, 128]])
            nc.sync.dma_start(out=dst, in_=contrib)

        # ---------------- snake ----------------
        with tc.tile_pool(name="snake", bufs=1) as sn, \
             tc.tile_pool(name="psumS", bufs=4, space="PSUM") as ppS, \
             tc.tile_pool(name="psumT", bufs=2, space="PSUM") as ppT:
            # init transpose-gather directly into circular-padded [ch, poly, 160]
            ipad_raw = sn.tile([128, 1, PADQ * PADV], bf16, tag="ipad", name="ipad")
            ccsrc = bass.AP(tensor=cc_in, offset=0, ap=[[128, NV], [1, 128]])
            nc.gpsimd.dma_gather(ipad_raw, ccsrc, iidx_t[:, :],
                                 PADQ * PADV, PADQ * PADV, 128, transpose=True,
                                 single_packet=False)
            ipad = ipad_raw[:, 0, :].rearrange("p (q k) -> p q k", k=PADV)

            spads = [sn.tile([128, PADQ, PADV], bf16, tag=f"spad{k}", name=f"spad{k}")
                     for k in range(8)]

            def circ_conv(dst_pad, src_pad, src_parts, lhsT_of_tap, bias_ap, gam_ap,
                          bet_ap, dilation, residual):
                for qb in range(NQB):
                    ps = ppS.tile([128, 512], f32, tag="psS", name="psS")
                    for t in range(9):
                        off = qb * 4 * PADV + 16 + (t - 4) * dilation
                        rhs = bass.AP(tensor=src_pad.tensor,
                                      offset=src_pad.offset + off,
                                      ap=[[src_pad.ap[0][0], src_parts],
                                          [PADV, 4], [1, 128]])
                        nc.tensor.matmul(ps, lhsT_of_tap(t), rhs,
                                         start=(t == 0), stop=(t == 8))
                    nc.scalar.activation(
                        dst_pad[:, qb * 4:(qb + 1) * 4, 16:144],
                        ps.rearrange("p (a b) -> p a b", a=4), AF.Relu, bias=bias_ap)
                ctr = dst_pad[:, :, 16:144]
                nc.vector.tensor_scalar(ctr, ctr, gam_ap, bet_ap,
                                        op0=ALU.mult, op1=ALU.add)
                if residual is not None:
                    nc.vector.tensor_tensor(ctr, ctr, residual[:, :, 16:144], ALU.add)
                nc.vector.tensor_copy(dst_pad[:, :, 0:16], dst_pad[:, :, 128:144])
                nc.vector.tensor_copy(dst_pad[:, :, 144:160], dst_pad[:, :, 16:32])

            circ_conv(spads[0], ipad[0:66], 66,
                      lambda t: headw_t[:, t, :],
                      headb_t[:, 0:1], headb_t[:, 1:2], headb_t[:, 2:3], 1, None)
            for i in range(NRES):
                circ_conv(spads[i + 1], spads[i], 128,
                          lambda t, i=i: resw_t[:, i * 9 + t, :],
                          resb_t[:, 0, i:i + 1], resb_t[:, 1, i:i + 1],
                          resb_t[:, 2, i:i + 1], DIL[i], spads[i])

            # fusion 1x1 (1024->256) + per-poly max over V (+ fus bias)
            gmax = [sn.tile([128, PADQ], f32, tag=f"gmax{m}", name=f"gmax{m}")
                    for m in range(2)]
            gb = [sn.tile([128, PADQ], bf16, tag=f"gb{m}", name=f"gb{m}")
                  for m in range(2)]
            for m in range(2):
                for qb in range(NQB):
                    ps = ppS.tile([128, 512], f32, tag="psS", name="psS")
                    for k in range(8):
                        sp = spads[k]
                        rhs = bass.AP(tensor=sp.tensor,
                                      offset=sp.offset + qb * 4 * PADV + 16,
                                      ap=[sp.ap[0], [PADV, 4], [1, 128]])
                        nc.tensor.matmul(ps, fusw_t[:, k, m * 128:(m + 1) * 128], rhs,
                                         start=(k == 0), stop=(k == 7))
                    nc.vector.tensor_reduce(gmax[m][:, qb * 4:(qb + 1) * 4],
                                            ps.rearrange("p (a b) -> p a b", a=4),
                                            axis=mybir.AxisListType.X, op=ALU.max)
                nc.vector.tensor_scalar(gb[m], gmax[m], fusb_t[:, m:m + 1], None,
                                        op0=ALU.add)

            # pred1: 1280 -> 256, relu
            h1 = [sn.tile([128, PADQ * 128], bf16, tag=f"h1_{m}", name=f"h1_{m}")
                  for m in range(2)]
            for m in range(2):
                for qb in range(NQB):
                    ps = ppS.tile([128, 512], f32, tag="psS", name="psS")
                    for k in range(10):
                        if k < 2:
                            rhs = _bcast(gb[k][:, qb * 4:(qb + 1) * 4], 128)
                        else:
                            sp = spads[k - 2]
                            rhs = bass.AP(tensor=sp.tensor,
                                          offset=sp.offset + qb * 4 * PADV + 16,
                                          ap=[sp.ap[0], [PADV, 4], [1, 128]])
                        nc.tensor.matmul(ps, pw1_t[:, k, m * 128:(m + 1) * 128], rhs,
                                         start=(k == 0), stop=(k == 9))
                    nc.scalar.activation(h1[m][:, qb * 512:(qb + 1) * 512], ps,
                                         AF.Relu, bias=pb1_t[:, m:m + 1])

            # pred2: 256 -> 64, relu
            h2 = sn.tile([64, PADQ * 128], bf16, tag="h2", name="h2")
            for qb in range(NQB):
                ps = ppT.tile([64, 512], f32, tag="psT", name="psT")
                for k in range(2):
                    nc.tensor.matmul(ps, pw2_t[:, k, :],
                                     h1[k][:, qb * 512:(qb + 1) * 512],
                                     start=(k == 0), stop=(k == 1))
                nc.scalar.activation(h2[:, qb * 512:(qb + 1) * 512], ps, AF.Relu,
                                     bias=pb2_t[:, 0:1])

            # pred3: 64 -> 2 per poly -> [128 v, PADQ, 2]
            ps3 = ppT.tile([128, PADQ * 2], f32, tag="psT3", name="psT3", bufs=1)
            for j in range(PADQ):
                nc.tensor.matmul(ps3[:, j * 2:(j + 1) * 2],
                                 h2[:, j * 128:(j + 1) * 128], pw3_t[:, :],
                                 start=True, stop=True)
            o_t = sn.tile([128, PADQ, 2], f32, tag="o_t", name="o_t")
            nc.vector.tensor_tensor(o_t, ps3.rearrange("p (a b) -> p a b", b=2),
                                    base_t, ALU.add)
            nc.sync.dma_start(out=d_out[:, :, :], in_=o_t)

    nc.compile()
    return nc


_NC_CACHE = {}


def _get_nc(P):
    if P not in _NC_CACHE:
        _NC_CACHE[P] = build_nc(P)
    return _NC_CACHE[P]


def _host_prep(inputs, P, counts, order, offs):
    """Build per-core in_maps."""
    cnn = np.asarray(inputs["cnn_feature"], np.float32)
    ipoly = np.asarray(inputs["i_it_poly"], np.float32)
    cpoly = np.asarray(inputs["c_it_poly"], np.float32)
    w1 = np.asarray(inputs["proj_w1"], np.float32)
    b2 = np.asarray(inputs["proj_b2"], np.float32)
    w2 = np.asarray(inputs["proj_w2"], np.float32)[:, :, 0, 0]  # [64, 256]
    NV = P * 128
    PADQ = -(-P // 4) * 4

    # ---- grid-sample host math (fp32, matches reference) ----
    ix = ipoly[..., 0] - np.float32(0.5)
    iy = ipoly[..., 1] - np.float32(0.5)
    x0 = np.floor(ix); y0 = np.floor(iy)
    wx = (ix - x0).astype(np.float32); wy = (iy - y0).astype(np.float32)
    x0i = x0.astype(np.int64); y0i = y0.astype(np.int64)
    corner_r = []; corner_w = []
    for dy, dx in ((0, 0), (0, 1), (1, 0), (1, 1)):
        xi = x0i + dx; yi = y0i + dy
        valid = (xi >= 0) & (xi < W) & (yi >= 0) & (yi < H)
        xc = np.clip(xi, 0, W - 1); yc = np.clip(yi, 0, H - 1)
        hw = yc * W + xc
        jt = hw // 128; p = hw % 128
        r = (jt // 8) * 1024 + p * 8 + (jt % 8)      # feat_dram row remap
        wgt = (wx if dx else (1 - wx)) * (wy if dy else (1 - wy))
        corner_r.append(r.astype(np.int64))
        corner_w.append((wgt * valid).astype(np.float32))
    s_v = np.sum(corner_w, axis=0)                    # [NP, V]

    # ---- shared packed weights ----
    w1p0 = np.zeros((128, 3, 2, 128), np.float32)
    w1p1 = np.zeros((70, 3, 2, 128), np.float32)
    for r0 in range(128):
        kh, ci = (0, r0) if r0 < 66 else (1, r0 - 66)
        for kw in range(3):
            for m in range(2):
                w1p0[r0, kw, m, :] = w1[m * 128:(m + 1) * 128, ci, kh, kw]
    for r1 in range(70):
        kh, ci = (1, 62 + r1) if r1 < 4 else (2, r1 - 4)
        for kw in range(3):
            for m in range(2):
                w1p1[r1, kw, m, :] = w1[m * 128:(m + 1) * 128, ci, kh, kw]
    w2t = np.transpose(w2, (1, 0)).reshape(2, 128, 64).transpose(1, 0, 2)

    headw = np.transpose(np.asarray(inputs["head_w"], np.float32), (1, 2, 0))
    headb = np.stack([np.asarray(inputs["head_b"], np.float32),
                      np.asarray(inputs["head_g"], np.float32),
                      np.asarray(inputs["head_bt"], np.float32)], axis=1)
    resw = np.transpose(np.asarray(inputs["res_w"], np.float32), (2, 0, 3, 1))
    resw = resw.reshape(128, 63, 128)
    resb = np.stack([np.asarray(inputs["res_b"], np.float32).T,
                     np.asarray(inputs["res_g"], np.float32).T,
                     np.asarray(inputs["res_bt"], np.float32).T], axis=1)
    fusw = np.transpose(np.asarray(inputs["fus_w"], np.float32).reshape(256, 8, 128),
                        (2, 1, 0))
    pw1 = np.transpose(np.asarray(inputs["pw1"], np.float32).reshape(256, 10, 128),
                       (2, 1, 0))
    pb1 = np.asarray(inputs["pb1"], np.float32).reshape(2, 128).T
    pw2 = np.transpose(np.asarray(inputs["pw2"], np.float32).reshape(64, 2, 128),
                       (2, 1, 0))
    pb2 = np.asarray(inputs["pb2"], np.float32).reshape(64, 1)
    pw3 = np.asarray(inputs["pw3"], np.float32).T
    pb3 = np.asarray(inputs["pb3"], np.float32)
    pb0 = np.asarray(inputs["proj_b1"], np.float32).reshape(2, 128).T
    fusb = np.asarray(inputs["fus_b"], np.float32).reshape(2, 128).T

    shared = {
        "w1p0": w1p0.astype(BF), "w1p1": w1p1.astype(BF), "w2t": w2t.astype(BF),
        "pb0": pb0, "fusb": fusb,
        "headw": headw.astype(BF), "headb": headb,
        "resw": resw.astype(BF), "resb": resb,
        "fusw": fusw.astype(BF), "pw1": pw1.astype(BF), "pb1": pb1,
        "pw2": pw2.astype(BF), "pb2": pb2, "pw3": pw3.astype(BF),
    }

    def pack16(idx_flat, cols):
        tab = np.zeros((16, cols), np.int16)
        n = len(idx_flat)
        tab[np.arange(n) % 16, np.arange(n) // 16] = idx_flat.astype(np.int16)
        return np.tile(tab, (8, 1))

    ind = np.asarray(inputs["ind"]).astype(np.int64)
    in_maps = []
    for c in range(N_CORES):
        img = cnn[c]
        img_pad = np.zeros((C_IN, PADW, PADW), np.float32)
        img_pad[:, 1:129, 1:129] = img
        flat = img_pad.reshape(C_IN, PIMG)
        stack0 = np.zeros((128, PIMG), np.float32)
        stack1 = np.zeros((70, PIMG), np.float32)
        stack0[0:66] = flat
        stack0[66:128, :PIMG - 130] = flat[0:62, 130:]
        stack1[0:4, :PIMG - 130] = flat[62:66, 130:]
        stack1[4:70, :PIMG - 260] = flat[0:66, 260:]

        own = order[offs[c]:offs[c + 1]]
        nown = len(own)
        idxc = np.zeros((4, NV), np.int64)
        wcomp = np.zeros((128, 4, P), np.float32)
        for cc in range(4):
            idxc[cc, :nown * 128] = corner_r[cc][own].reshape(-1)
            wcomp[:, cc, :nown] = corner_w[cc][own].T
        b2s = np.zeros((128, P, 64), np.float32)
        b2s[:, :nown, :] = s_v[own].T[:, :, None] * b2[None, None, :]
        coords = np.zeros((128, P, 2), np.float32)
        coords[:, :nown, :] = (cpoly[own] * RO).transpose(1, 0, 2)

        iidx = np.zeros(PADQ * PADV, np.int64)
        kk = np.arange(PADV)
        for q in range(nown):
            iidx[q * PADV:(q + 1) * PADV] = q * 128 + (kk + 112) % 128
        base = np.zeros((128, PADQ, 2), np.float32)
        if nown:
            base[:, :nown, :] = (ipoly[own] * RO + pb3[None, None, :]) \
                .transpose(1, 0, 2).astype(np.float32)

        m = {
            "stack0": stack0.astype(BF), "stack1": stack1.astype(BF),
            "idxc": np.stack([pack16(idxc[cc], NV // 16) for cc in range(4)], axis=1),
            "wcomp": wcomp, "b2s": b2s, "coords": coords.astype(BF),
            "iidx": pack16(iidx, PADQ * PADV // 16),
            "base": base,
        }
        m.update(shared)
        in_maps.append(m)
    return in_maps


def kernel(**inputs):
    ind = np.asarray(inputs["ind"]).astype(np.int64)
    counts = np.bincount(ind, minlength=N_CORES)
    P = int(counts.max())
    assert P <= 31, f"per-image poly count {P} exceeds int16 gather range"
    order = np.argsort(ind, kind="stable")
    offs = np.concatenate([[0], np.cumsum(counts)])

    nc = _get_nc(P)
    in_maps = _host_prep(inputs, P, counts, order, offs)
    res = run_bass_kernel_spmd(nc, in_maps, list(range(N_CORES)))

    out = np.zeros((NP, V, 2), np.float32)
    for c in range(N_CORES):
        oc = res.results[c]["out"]  # [128v, PADQ, 2]
        own = order[offs[c]:offs[c + 1]]
        for q, opoly in enumerate(own):
            out[opoly] = oc[:, q, :]
    return out


# revision 15
# speedup vs baseline: 1.0394x; 1.0394x over previous
"""Trainium2 Bass kernel for nn_Evolution_26697516712465 (deep-snake GNN).

Self-contained: takes FULL inputs, shards batch across 8 NeuronCores internally
(one image per core; each core runs the snake for the polys of its own image),
returns FULL output [128, 128, 2] fp32.
"""
import numpy as np
import ml_dtypes
from contextlib import ExitStack

import concourse.bass as bass
import concourse.bacc as bacc
import concourse.mybir as mybir
import concourse.tile as tile
from concourse.library_config import mlp as mlp_lib
from concourse.bass_utils import run_bass_kernel_spmd

N_CORES = 8
B, C_IN, H, W = 8, 66, 128, 128
NP, V = 128, 128
RO = 4.0
DIL = (1, 1, 1, 2, 2, 4, 4)
NRES = 7
HW = H * W          # 16384
PADW = W + 2        # 130
PIMG = PADW * PADW  # 16900
PADV = 160          # 16 + 128 + 16 circular pad

f32 = mybir.dt.float32
bf16 = mybir.dt.bfloat16
i16 = mybir.dt.int16
AF = mybir.ActivationFunctionType
ALU = mybir.AluOpType

BF = ml_dtypes.bfloat16


def _bcast(ap_obj, n):
    """Append a step-0 (broadcast) innermost free dim of size n to an AP."""
    return bass.AP(tensor=ap_obj.tensor, offset=ap_obj.offset,
                   ap=[*ap_obj.ap, [0, n]])


def build_nc(P):
    """Build the SPMD Bass program. P = max polys per image."""
    nc = bacc.Bacc("TRN2", target_bir_lowering=False, debug=False)
    NV = P * 128           # corner-gather idx count (multiple of 128)
    ICOLS = NV // 16
    PADQ = -(-P // 4) * 4  # snake poly slots (multiple of 4)
    NQB = PADQ // 4

    # ---------------- inputs ----------------
    d_stack0 = nc.declare_dram_parameter("stack0", [128, PIMG], bf16, isOutput=False)
    d_stack1 = nc.declare_dram_parameter("stack1", [70, PIMG], bf16, isOutput=False)
    d_w1p0 = nc.declare_dram_parameter("w1p0", [128, 3, 2, 128], bf16, isOutput=False)
    d_w1p1 = nc.declare_dram_parameter("w1p1", [70, 3, 2, 128], bf16, isOutput=False)
    d_w2t = nc.declare_dram_parameter("w2t", [128, 2, 64], bf16, isOutput=False)
    d_pb0 = nc.declare_dram_parameter("pb0", [128, 2], f32, isOutput=False)
    d_fusb = nc.declare_dram_parameter("fusb", [128, 2], f32, isOutput=False)
    d_idxc = nc.declare_dram_parameter("idxc", [128, 4, ICOLS], i16, isOutput=False)
    d_wcomp = nc.declare_dram_parameter("wcomp", [128, 4, P], f32, isOutput=False)
    d_b2s = nc.declare_dram_parameter("b2s", [128, P, 64], f32, isOutput=False)
    d_coords = nc.declare_dram_parameter("coords", [128, P, 2], bf16, isOutput=False)
    d_iidx = nc.declare_dram_parameter("iidx", [128, PADQ * PADV // 16], i16, isOutput=False)
    d_base = nc.declare_dram_parameter("base", [128, PADQ, 2], f32, isOutput=False)
    d_headw = nc.declare_dram_parameter("headw", [66, 9, 128], bf16, isOutput=False)
    d_headb = nc.declare_dram_parameter("headb", [128, 3], f32, isOutput=False)
    d_resw = nc.declare_dram_parameter("resw", [128, 63, 128], bf16, isOutput=False)
    d_resb = nc.declare_dram_parameter("resb", [128, 3, 7], f32, isOutput=False)
    d_fusw = nc.declare_dram_parameter("fusw", [128, 8, 256], bf16, isOutput=False)
    d_pw1 = nc.declare_dram_parameter("pw1", [128, 10, 256], bf16, isOutput=False)
    d_pb1 = nc.declare_dram_parameter("pb1", [128, 2], f32, isOutput=False)
    d_pw2 = nc.declare_dram_parameter("pw2", [128, 2, 64], bf16, isOutput=False)
    d_pb2 = nc.declare_dram_parameter("pb2", [64, 1], f32, isOutput=False)
    d_pw3 = nc.declare_dram_parameter("pw3", [64, 2], bf16, isOutput=False)
    d_out = nc.declare_dram_parameter("out", [128, PADQ, 2], f32, isOutput=True)

    feat_dram = nc.dram_tensor("feat_dram", [HW, 64], f32)
    cc_in = nc.dram_tensor("cc_in", [NV, 128], bf16)

    with tile.TileContext(nc, num_cores=N_CORES) as tc, ExitStack() as top:
        wpool = top.enter_context(tc.tile_pool(name="weights", bufs=1))
        w2t_t = wpool.tile([128, 2, 64], bf16)
        nc.sync.dma_start(out=w2t_t, in_=d_w2t[:, :, :])
        pb0_t = wpool.tile([128, 2], f32)
        nc.sync.dma_start(out=pb0_t, in_=d_pb0[:, :])
        fusb_t = wpool.tile([128, 2], f32)
        nc.sync.dma_start(out=fusb_t, in_=d_fusb[:, :])
        idxc_t = wpool.tile([128, 4, ICOLS], i16)
        nc.sync.dma_start(out=idxc_t, in_=d_idxc[:, :, :])
        wcomp_t = wpool.tile([128, 4, P], f32)
        nc.sync.dma_start(out=wcomp_t, in_=d_wcomp[:, :, :])
        b2s_t = wpool.tile([128, P, 64], f32)
        nc.sync.dma_start(out=b2s_t, in_=d_b2s[:, :, :])
        coords_t = wpool.tile([128, P, 2], bf16)
        nc.sync.dma_start(out=coords_t, in_=d_coords[:, :, :])
        iidx_t = wpool.tile([128, PADQ * PADV // 16], i16)
        nc.sync.dma_start(out=iidx_t, in_=d_iidx[:, :])
        base_t = wpool.tile([128, PADQ, 2], f32)
        nc.sync.dma_start(out=base_t, in_=d_base[:, :, :])
        headw_t = wpool.tile([66, 9, 128], bf16)
        headb_t = wpool.tile([128, 3], f32)
        resw_t = wpool.tile([128, 63, 128], bf16)
        resb_t = wpool.tile([128, 3, 7], f32)
        fusw_t = wpool.tile([128, 8, 256], bf16)
        pw1_t = wpool.tile([128, 10, 256], bf16)
        pb1_t = wpool.tile([128, 2], f32)
        pw2_t = wpool.tile([128, 2, 64], bf16)
        pb2_t = wpool.tile([64, 1], f32)
        pw3_t = wpool.tile([64, 2], bf16)

        nc.gpsimd.load_library(mlp_lib)

        # relu1 lives across conv1 + conv2
        with tc.tile_pool(name="relu1", bufs=1) as rpool:
            r1 = [rpool.tile([128, HW], bf16, tag=f"r1_{m}", name=f"r1_{m}")
                  for m in range(2)]

            # ------------ conv1: 3x3 66->256 (bf16, K packed 128+70) ------------
            with tc.tile_pool(name="stacks", bufs=1) as stpool, \
                 tc.tile_pool(name="psumA", bufs=3, space="PSUM") as ppA:
                st0 = stpool.tile([128, PIMG], bf16)
                HALF = 68 * PADW
                nc.sync.dma_start(out=st0[:, :HALF], in_=d_stack0[:, :HALF])
                nc.sync.dma_start(out=st0[:, HALF:], in_=d_stack0[:, HALF:])
                st1 = stpool.tile([70, PIMG], bf16)
                nc.sync.dma_start(out=st1[:, :HALF], in_=d_stack1[:, :HALF])
                nc.sync.dma_start(out=st1[:, HALF:], in_=d_stack1[:, HALF:])
                w1p0_t = stpool.tile([128, 3, 2, 128], bf16)
                nc.sync.dma_start(out=w1p0_t, in_=d_w1p0[:, :, :, :])
                w1p1_t = stpool.tile([70, 3, 2, 128], bf16)
                nc.sync.dma_start(out=w1p1_t, in_=d_w1p1[:, :, :, :])

                for t in range(32):          # hw tiles of 512 = 4 image rows
                    y0 = 4 * t
                    for m in range(2):       # out-channel half
                        ps = ppA.tile([128, 512], f32, tag="psA", name="psA")
                        i = 0
                        for (stk, wt) in ((st0, w1p0_t), (st1, w1p1_t)):
                            for kw in range(3):
                                rhs = bass.AP(tensor=stk.tensor,
                                              offset=stk.offset + y0 * PADW + kw,
                                              ap=[stk.ap[0], [PADW, 4], [1, 128]])
                                nc.tensor.matmul(ps, wt[:, kw, m, :], rhs,
                                                 start=(i == 0), stop=(i == 5))
                                i += 1
                        nc.scalar.activation(r1[m][:, t * 512:(t + 1) * 512], ps,
                                             AF.Relu, bias=pb0_t[:, m:m + 1])

            # ------------ conv2: 1x1 256->64, out [hw, 64] fp32 -> DRAM ------------
            with tc.tile_pool(name="psumB", bufs=2, space="PSUM") as ppB, \
                 tc.tile_pool(name="stage", bufs=3) as spool:
                for g in range(16):
                    ps2 = ppB.tile([128, 512], f32, tag="psB", name="psB")
                    for j in range(8):
                        hw0 = (g * 8 + j) * 128
                        for ch in range(2):
                            nc.tensor.matmul(ps2[:, j * 64:(j + 1) * 64],
                                             r1[ch][:, hw0:hw0 + 128],
                                             w2t_t[:, ch, :],
                                             start=(ch == 0), stop=(ch == 1))
                    stg = spool.tile([128, 512], f32, tag="stage", name="stg")
                    nc.vector.tensor_copy(stg, ps2)
                    dst = bass.AP(tensor=feat_dram, offset=g * 65536,
                                  ap=[[512, 128], [1, 512]])
                    nc.sync.dma_start(out=dst, in_=stg)

        # ------------ bilinear gather + weighted sum + vertex rows ------------
        with tc.tile_pool(name="gpool", bufs=1) as gpool:
            gts = []
            for c in range(4):
                gt = gpool.tile([128, P, 64], f32, tag=f"g{c}", name=f"g{c}")
                src = bass.AP(tensor=feat_dram, offset=0, ap=[[64, HW], [1, 64]])
                nc.gpsimd.dma_gather(gt, src, idxc_t[:, c, :], NV, NV, 64,
                                     single_packet=False)
                gts.append(gt)
            vert = gpool.tile([128, P, 64], f32, tag="vert", name="vert")
            tmp = gpool.tile([128, P, 64], f32, tag="tmp", name="tmp")
            for c in range(4):
                wb = _bcast(wcomp_t[:, c, :], 64)
                if c == 0:
                    nc.vector.tensor_tensor(vert, gts[c], wb, ALU.mult)
                else:
                    nc.vector.tensor_tensor(tmp, gts[c], wb, ALU.mult)
                    nc.vector.tensor_tensor(vert, vert, tmp, ALU.add)
            nc.vector.tensor_tensor(vert, vert, b2s_t, ALU.add)

            contrib = gpool.tile([128, P, 128], bf16, tag="contrib", name="contrib")
            nc.vector.memset(contrib, 0.0)
            nc.vector.tensor_copy(contrib[:, :, 0:64], vert)
            nc.vector.tensor_copy(contrib[:, :, 64:66], coords_t)
            # SBUF [v, q, ch] -> DRAM row q*128+v
            dst = bass.AP(tensor=cc_in, offset=0,
                          ap=[[128, 128], [128 * 128, P], [1, 128]])
            nc.sync.dma_start(out=dst, in_=contrib)

        # snake weights load late (off conv1's critical DMA path)
        nc.sync.dma_start(out=headw_t, in_=d_headw[:, :, :])
        nc.sync.dma_start(out=headb_t, in_=d_headb[:, :])
        nc.sync.dma_start(out=resw_t, in_=d_resw[:, :, :])
        nc.sync.dma_start(out=resb_t, in_=d_resb[:, :, :])
        nc.sync.dma_start(out=fusw_t, in_=d_fusw[:, :, :])
        nc.sync.dma_start(out=pw1_t, in_=d_pw1[:, :, :])
        nc.sync.dma_start(out=pb1_t, in_=d_pb1[:, :])
        nc.sync.dma_start(out=pw2_t, in_=d_pw2[:, :, :])
        nc.sync.dma_start(out=pb2_t, in_=d_pb2[:, :])
        nc.sync.dma_start(out=pw3_t, in_=d_pw3[:, :])

        # ---------------- snake ----------------
        with tc.tile_pool(name="snake", bufs=1) as sn, \
             tc.tile_pool(name="psumS", bufs=4, space="PSUM") as ppS, \
             tc.tile_pool(name="psumT", bufs=2, space="PSUM") as ppT:
            # init transpose-gather directly into circular-padded [ch, poly, 160]
            ipad_raw = sn.tile([128, 1, PADQ * PADV], bf16, tag="ipad", name="ipad")
            ccsrc = bass.AP(tensor=cc_in, offset=0, ap=[[128, NV], [1, 128]])
            nc.gpsimd.dma_gather(ipad_raw, ccsrc, iidx_t[:, :],
                                 PADQ * PADV, PADQ * PADV, 128, transpose=True,
                                 single_packet=False)
            ipad = ipad_raw[:, 0, :].rearrange("p (q k) -> p q k", k=PADV)

            spads = [sn.tile([128, PADQ, PADV], bf16, tag=f"spad{k}", name=f"spad{k}")
                     for k in range(8)]

            def circ_conv(dst_pad, src_pad, src_parts, lhsT_of_tap, bias_ap, gam_ap,
                          bet_ap, dilation, residual):
                for qb in range(NQB):
                    ps = ppS.tile([128, 512], f32, tag="psS", name="psS")
                    for t in range(9):
                        off = qb * 4 * PADV + 16 + (t - 4) * dilation
                        rhs = bass.AP(tensor=src_pad.tensor,
                                      offset=src_pad.offset + off,
                                      ap=[[src_pad.ap[0][0], src_parts],
                                          [PADV, 4], [1, 128]])
                        nc.tensor.matmul(ps, lhsT_of_tap(t), rhs,
                                         start=(t == 0), stop=(t == 8))
                    nc.scalar.activation(
                        dst_pad[:, qb * 4:(qb + 1) * 4, 16:144],
                        ps.rearrange("p (a b) -> p a b", a=4), AF.Relu, bias=bias_ap)
                ctr = dst_pad[:, :, 16:144]
                nc.vector.tensor_scalar(ctr, ctr, gam_ap, bet_ap,
                                        op0=ALU.mult, op1=ALU.add)
                if residual is not None:
                    nc.vector.tensor_tensor(ctr, ctr, residual[:, :, 16:144], ALU.add)
                nc.vector.tensor_copy(dst_pad[:, :, 0:16], dst_pad[:, :, 128:144])
                nc.vector.tensor_copy(dst_pad[:, :, 144:160], dst_pad[:, :, 16:32])

            circ_conv(spads[0], ipad[0:66], 66,
                      lambda t: headw_t[:, t, :],
                      headb_t[:, 0:1], headb_t[:, 1:2], headb_t[:, 2:3], 1, None)
            for i in range(NRES):
                circ_conv(spads[i + 1], spads[i], 128,
                          lambda t, i=i: resw_t[:, i * 9 + t, :],
                          resb_t[:, 0, i:i + 1], resb_t[:, 1, i:i + 1],
                          resb_t[:, 2, i:i + 1], DIL[i], spads[i])

            # fusion 1x1 (1024->256) + per-poly max over V (+ fus bias)
            gmax = [sn.tile([128, PADQ], f32, tag=f"gmax{m}", name=f"gmax{m}")
                    for m in range(2)]
            gb = [sn.tile([128, PADQ], bf16, tag=f"gb{m}", name=f"gb{m}")
                  for m in range(2)]
            for m in range(2):
                for qb in range(NQB):
                    ps = ppS.tile([128, 512], f32, tag="psS", name="psS")
                    for k in range(8):
                        sp = spads[k]
                        rhs = bass.AP(tensor=sp.tensor,
                                      offset=sp.offset + qb * 4 * PADV + 16,
                                      ap=[sp.ap[0], [PADV, 4], [1, 128]])
                        nc.tensor.matmul(ps, fusw_t[:, k, m * 128:(m + 1) * 128], rhs,
                                         start=(k == 0), stop=(k == 7))
                    nc.vector.tensor_reduce(gmax[m][:, qb * 4:(qb + 1) * 4],
                                            ps.rearrange("p (a b) -> p a b", a=4),
                                            axis=mybir.AxisListType.X, op=ALU.max)
                nc.vector.tensor_scalar(gb[m], gmax[m], fusb_t[:, m:m + 1], None,
                                        op0=ALU.add)

            # pred1: 1280 -> 256, relu
            h1 = [sn.tile([128, PADQ * 128], bf16, tag=f"h1_{m}", name=f"h1_{m}")
                  for m in range(2)]
            for m in range(2):
                for qb in range(NQB):
                    ps = ppS.tile([128, 512], f32, tag="psS", name="psS")
                    for k in range(10):
                        if k < 2:
                            rhs = _bcast(gb[k][:, qb * 4:(qb + 1) * 4], 128)
                        else:
                            sp = spads[k - 2]
                            rhs = bass.AP(tensor=sp.tensor,
                                          offset=sp.offset + qb * 4 * PADV + 16,
                                          ap=[sp.ap[0], [PADV, 4], [1, 128]])
                        nc.tensor.matmul(ps, pw1_t[:, k, m * 128:(m + 1) * 128], rhs,
                                         start=(k == 0), stop=(k == 9))
                    nc.scalar.activation(h1[m][:, qb * 512:(qb + 1) * 512], ps,
                                         AF.Relu, bias=pb1_t[:, m:m + 1])

            # pred2: 256 -> 64, relu
            h2 = sn.tile([64, PADQ * 128], bf16, tag="h2", name="h2")
            for qb in range(NQB):
                ps = ppT.tile([64, 512], f32, tag="psT", name="psT")
                for k in range(2):
                    nc.tensor.matmul(ps, pw2_t[:, k, :],
                                     h1[k][:, qb * 512:(qb + 1) * 512],
                                     start=(k == 0), stop=(k == 1))
                nc.scalar.activation(h2[:, qb * 512:(qb + 1) * 512], ps, AF.Relu,
                                     bias=pb2_t[:, 0:1])

            # pred3: 64 -> 2 per poly -> [128 v, PADQ, 2]
            ps3 = ppT.tile([128, PADQ * 2], f32, tag="psT3", name="psT3", bufs=1)
            for j in range(PADQ):
                nc.tensor.matmul(ps3[:, j * 2:(j + 1) * 2],
                                 h2[:, j * 128:(j + 1) * 128], pw3_t[:, :],
                                 start=True, stop=True)
            o_t = sn.tile([128, PADQ, 2], f32, tag="o_t", name="o_t")
            nc.vector.tensor_tensor(o_t, ps3.rearrange("p (a b) -> p a b", b=2),
                                    base_t, ALU.add)
            nc.sync.dma_start(out=d_out[:, :, :], in_=o_t)

    nc.compile()
    return nc


_NC_CACHE = {}


def _get_nc(P):
    if P not in _NC_CACHE:
        _NC_CACHE[P] = build_nc(P)
    return _NC_CACHE[P]


def _host_prep(inputs, P, counts, order, offs):
    """Build per-core in_maps."""
    cnn = np.asarray(inputs["cnn_feature"], np.float32)
    ipoly = np.asarray(inputs["i_it_poly"], np.float32)
    cpoly = np.asarray(inputs["c_it_poly"], np.float32)
    w1 = np.asarray(inputs["proj_w1"], np.float32)
    b2 = np.asarray(inputs["proj_b2"], np.float32)
    w2 = np.asarray(inputs["proj_w2"], np.float32)[:, :, 0, 0]  # [64, 256]
    NV = P * 128
    PADQ = -(-P // 4) * 4

    # ---- grid-sample host math (fp32, matches reference) ----
    ix = ipoly[..., 0] - np.float32(0.5)
    iy = ipoly[..., 1] - np.float32(0.5)
    x0 = np.floor(ix); y0 = np.floor(iy)
    wx = (ix - x0).astype(np.float32); wy = (iy - y0).astype(np.float32)
    x0i = x0.astype(np.int64); y0i = y0.astype(np.int64)
    corner_r = []; corner_w = []
    for dy, dx in ((0, 0), (0, 1), (1, 0), (1, 1)):
        xi = x0i + dx; yi = y0i + dy
        valid = (xi >= 0) & (xi < W) & (yi >= 0) & (yi < H)
        xc = np.clip(xi, 0, W - 1); yc = np.clip(yi, 0, H - 1)
        hw = yc * W + xc
        jt = hw // 128; p = hw % 128
        r = (jt // 8) * 1024 + p * 8 + (jt % 8)      # feat_dram row remap
        wgt = (wx if dx else (1 - wx)) * (wy if dy else (1 - wy))
        corner_r.append(r.astype(np.int64))
        corner_w.append((wgt * valid).astype(np.float32))
    s_v = np.sum(corner_w, axis=0)                    # [NP, V]

    # ---- shared packed weights ----
    w1p0 = np.zeros((128, 3, 2, 128), np.float32)
    w1p1 = np.zeros((70, 3, 2, 128), np.float32)
    for r0 in range(128):
        kh, ci = (0, r0) if r0 < 66 else (1, r0 - 66)
        for kw in range(3):
            for m in range(2):
                w1p0[r0, kw, m, :] = w1[m * 128:(m + 1) * 128, ci, kh, kw]
    for r1 in range(70):
        kh, ci = (1, 62 + r1) if r1 < 4 else (2, r1 - 4)
        for kw in range(3):
            for m in range(2):
                w1p1[r1, kw, m, :] = w1[m * 128:(m + 1) * 128, ci, kh, kw]
    w2t = np.transpose(w2, (1, 0)).reshape(2, 128, 64).transpose(1, 0, 2)

    headw = np.transpose(np.asarray(inputs["head_w"], np.float32), (1, 2, 0))
    headb = np.stack([np.asarray(inputs["head_b"], np.float32),
                      np.asarray(inputs["head_g"], np.float32),
                      np.asarray(inputs["head_bt"], np.float32)], axis=1)
    resw = np.transpose(np.asarray(inputs["res_w"], np.float32), (2, 0, 3, 1))
    resw = resw.reshape(128, 63, 128)
    resb = np.stack([np.asarray(inputs["res_b"], np.float32).T,
                     np.asarray(inputs["res_g"], np.float32).T,
                     np.asarray(inputs["res_bt"], np.float32).T], axis=1)
    fusw = np.transpose(np.asarray(inputs["fus_w"], np.float32).reshape(256, 8, 128),
                        (2, 1, 0))
    pw1 = np.transpose(np.asarray(inputs["pw1"], np.float32).reshape(256, 10, 128),
                       (2, 1, 0))
    pb1 = np.asarray(inputs["pb1"], np.float32).reshape(2, 128).T
    pw2 = np.transpose(np.asarray(inputs["pw2"], np.float32).reshape(64, 2, 128),
                       (2, 1, 0))
    pb2 = np.asarray(inputs["pb2"], np.float32).reshape(64, 1)
    pw3 = np.asarray(inputs["pw3"], np.float32).T
    pb3 = np.asarray(inputs["pb3"], np.float32)
    pb0 = np.asarray(inputs["proj_b1"], np.float32).reshape(2, 128).T
    fusb = np.asarray(inputs["fus_b"], np.float32).reshape(2, 128).T

    shared = {
        "w1p0": w1p0.astype(BF), "w1p1": w1p1.astype(BF), "w2t": w2t.astype(BF),
        "pb0": pb0, "fusb": fusb,
        "headw": headw.astype(BF), "headb": headb,
        "resw": resw.astype(BF), "resb": resb,
        "fusw": fusw.astype(BF), "pw1": pw1.astype(BF), "pb1": pb1,
        "pw2": pw2.astype(BF), "pb2": pb2, "pw3": pw3.astype(BF),
    }

    def pack16(idx_flat, cols):
        tab = np.zeros((16, cols), np.int16)
        n = len(idx_flat)
        tab[np.arange(n) % 16, np.arange(n) // 16] = idx_flat.astype(np.int16)
        return np.tile(tab, (8, 1))

    ind = np.asarray(inputs["ind"]).astype(np.int64)
    in_maps = []
    for c in range(N_CORES):
        img = cnn[c]
        img_pad = np.zeros((C_IN, PADW, PADW), np.float32)
        img_pad[:, 1:129, 1:129] = img
        flat = img_pad.reshape(C_IN, PIMG)
        stack0 = np.zeros((128, PIMG), np.float32)
        stack1 = np.zeros((70, PIMG), np.float32)
        stack0[0:66] = flat
        stack0[66:128, :PIMG - 130] = flat[0:62, 130:]
        stack1[0:4, :PIMG - 130] = flat[62:66, 130:]
        stack1[4:70, :PIMG - 260] = flat[0:66, 260:]

        own = order[offs[c]:offs[c + 1]]
        nown = len(own)
        idxc = np.zeros((4, NV), np.int64)
        wcomp = np.zeros((128, 4, P), np.float32)
        for cc in range(4):
            idxc[cc, :nown * 128] = corner_r[cc][own].reshape(-1)
            wcomp[:, cc, :nown] = corner_w[cc][own].T
        b2s = np.zeros((128, P, 64), np.float32)
        b2s[:, :nown, :] = s_v[own].T[:, :, None] * b2[None, None, :]
        coords = np.zeros((128, P, 2), np.float32)
        coords[:, :nown, :] = (cpoly[own] * RO).transpose(1, 0, 2)

        iidx = np.zeros(PADQ * PADV, np.int64)
        kk = np.arange(PADV)
        for q in range(nown):
            iidx[q * PADV:(q + 1) * PADV] = q * 128 + (kk + 112) % 128
        base = np.zeros((128, PADQ, 2), np.float32)
        if nown:
            base[:, :nown, :] = (ipoly[own] * RO + pb3[None, None, :]) \
                .transpose(1, 0, 2).astype(np.float32)

        m = {
            "stack0": stack0.astype(BF), "stack1": stack1.astype(BF),
            "idxc": np.stack([pack16(idxc[cc], NV // 16) for cc in range(4)], axis=1),
            "wcomp": wcomp, "b2s": b2s, "coords": coords.astype(BF),
            "iidx": pack16(iidx, PADQ * PADV // 16),
            "base": base,
        }
        m.update(shared)
        in_maps.append(m)
    return in_maps


def kernel(**inputs):
    ind = np.asarray(inputs["ind"]).astype(np.int64)
    counts = np.bincount(ind, minlength=N_CORES)
    P = int(counts.max())
    assert P <= 31, f"per-image poly count {P} exceeds int16 gather range"
    order = np.argsort(ind, kind="stable")
    offs = np.concatenate([[0], np.cumsum(counts)])

    nc = _get_nc(P)
    in_maps = _host_prep(inputs, P, counts, order, offs)
    res = run_bass_kernel_spmd(nc, in_maps, list(range(N_CORES)))

    out = np.zeros((NP, V, 2), np.float32)
    for c in range(N_CORES):
        oc = res.results[c]["out"]  # [128v, PADQ, 2]
        own = order[offs[c]:offs[c + 1]]
        for q, opoly in enumerate(own):
            out[opoly] = oc[:, q, :]
    return out
